# revision 9
# baseline (speedup 1.0000x reference)
"""Bidirectional H=1 LSTM attention kernel for Trainium2 (8 NeuronCores).

Model: hs = BiLSTM(x) [B,T,2] -> att = softmax(mean(hs,-1), axis=T) -> out = att[:,:,None]*x
Shapes: B=32, T=4096, E=300, H=1.

v2: fp16 datapath (memory-regime problem; halves HBM traffic, 4x PE rate):
  - Host converts x to fp16 e-major [300, 16384] per core; out returned fp16
    and upcast on host.  All on-chip tensors fp16 except PSUM (f32), scan
    internal state (f32) and the softmax sum (f32).
  - Phase 1: stream xT blocks (2048 toks), PE computes xg [8,2048] in PSUM
    (fp16 matmuls, 1 cyc/row), ACT evacuates +bias -> fp16 staging, DVE
    flips the bwd rows (time-reversed storage), DMAs -> dxg DRAM.
  - Phase 2: per-(d,b) merged gather DMAs (16 chunk-rows each) -> scan
    layout xg_tile [128 rows = (d,b,k) b-major, 4 gates x S cols].
  - Phase 3: fixed-point halo scan (L=256, W=32, N_ITER iters), fp16 ops
    (DVE 2x packing), fp32 scan state inside tensor_tensor_scan.
  - Phase 4: softmax over T per batch via PE sel-reduction.
  - Phase 5: att broadcast to 128 partitions via PE outer product (no HBM
    broadcast traffic), DVE/gpsimd multiply, fp16 writes.
"""

import sys

sys.path.insert(0, "/opt/trn_rl_repo")

import numpy as np
from contextlib import ExitStack

import concourse.bass as bass
import concourse.bacc as bacc
import concourse.tile as tile
from concourse import mybir
from concourse.bass_utils import run_bass_kernel_spmd

F32 = mybir.dt.float32
F16 = mybir.dt.float16
AF = mybir.ActivationFunctionType
ALU = mybir.AluOpType

NCORES = 8
B, T, E = 32, 4096, 300
BL = B // NCORES          # batches per core
TOK = BL * T              # tokens per core (b-major)
L, W = 256, 32            # chunk len, halo warmup
S = L + W                 # scan steps per chunk
K = T // L                # chunks per (dir, batch)
P = 128                   # partitions = d*64 + b*16 + k (b-major)
N_ITER = 4                # fixed-point iterations (validated offline:
                          # N=4 hits the fp16 error floor 2.3e-3; N=3 7e-3)
PADROW = W + T + W        # padded xg row: [0..W) zeros, [W..W+T) data, tail zeros
CB = 2048                 # phase-1/5 token block
NB = TOK // CB
# gate order inside a block row: (i, f, o, g) ; pytorch order is (i, f, g, o)
GATE_PERM = [0, 1, 3, 2]


def _build_nc():
    nc = bacc.Bacc(None, target_bir_lowering=False, debug=False)
    xT = nc.declare_dram_parameter("xT", [E, TOK], F16, isOutput=False)
    w8T = nc.declare_dram_parameter("w8T", [E, 36], F16, isOutput=False)
    b8 = nc.declare_dram_parameter("b8", [36, 1], F32, isOutput=False)
    whh = nc.declare_dram_parameter("whh", [P, 4], F32, isOutput=False)
    sel = nc.declare_dram_parameter("sel", [64, 4], F32, isOutput=False)
    selT = nc.declare_dram_parameter("selT", [4, 64], F32, isOutput=False)
    permM = nc.declare_dram_parameter("permM", [64, 64], F16, isOutput=False)
    outT = nc.declare_dram_parameter("outT", [E, TOK], F16, isOutput=True)

    # internal DRAM scratch: rows b*8 + d*4 + g
    dxg = nc.dram_tensor("dxg", [32, PADROW], F16)

    with tile.TileContext(nc) as tc, ExitStack() as ctx:
        singles = ctx.enter_context(tc.tile_pool(name="singles", bufs=1))
        p1ctx = ExitStack()
        stage = p1ctx.enter_context(tc.tile_pool(name="stage", bufs=2))
        psA = p1ctx.enter_context(tc.tile_pool(name="psA", bufs=2, space="PSUM"))

        # ---- constants / resident tiles ----
        w8a = singles.tile([128, 36], F16)
        w8b = singles.tile([128, 36], F16)
        w8c = singles.tile([44, 36], F16)
        nc.gpsimd.dma_start(out=w8a, in_=w8T[0:128, :])
        nc.gpsimd.dma_start(out=w8b, in_=w8T[128:256, :])
        nc.gpsimd.dma_start(out=w8c, in_=w8T[256:300, :])
        b8_sb = singles.tile([36, 1], F32)
        nc.sync.dma_start(out=b8_sb, in_=b8[:, :])
        whh_sb = singles.tile([P, 4], F32)
        nc.sync.dma_start(out=whh_sb, in_=whh[:, :])
        sel_sb = singles.tile([64, 4], F32)
        nc.sync.dma_start(out=sel_sb, in_=sel[:, :])
        selT_sb = singles.tile([4, 64], F32)
        nc.sync.dma_start(out=selT_sb, in_=selT[:, :])
        perm_sb = singles.tile([64, 64], F16)
        nc.sync.dma_start(out=perm_sb, in_=permM[:, :])
        ones1 = singles.tile([1, 128], F16)
        nc.vector.memset(ones1[:, :], 1.0)

        xT0 = singles.tile([128, TOK], F16)   # e 0..127 resident
        xT1 = singles.tile([128, TOK], F16)   # e 128..255 resident
        xT2 = singles.tile([44, TOK], F16)    # e 256..299 resident

        # zero-pad regions of dxg (halo reads beyond sequence ends)
        zpad = singles.tile([32, W], F16)
        nc.vector.memset(zpad[:, :], 0.0)
        nc.sync.dma_start(out=dxg[:, 0:W], in_=zpad[:, :])
        nc.sync.dma_start(out=dxg[:, W + T:PADROW], in_=zpad[:, :])

        # scan-layout xg: rows p = d*64 + b*16 + k, cols g*S + s
        xg_tile = singles.tile([128, 4 * S], F16)

        # ---- phase 1: stream xT, compute xg -> dxg; gather per (d,b) ----
        for tt in range(NB):
            b = (tt * CB) // T
            toff = (tt * CB) % T
            cols = slice(tt * CB, (tt + 1) * CB)
            d0 = nc.gpsimd.dma_start(out=xT0[:, cols], in_=xT[0:128, cols])
            d1 = nc.gpsimd.dma_start(out=xT1[:, cols], in_=xT[128:256, cols])
            d2 = nc.gpsimd.dma_start(out=xT2[:, cols], in_=xT[256:300, cols])
            ps = psA.tile([36, CB], F32, tag="ps")
            # Matmult codegen has a single sync-wait slot; absorb each fresh
            # DMA's semaphore with a tiny touch matmul reading only that
            # tensor, so real matmuls only wait on the PSUM WAR (ACT evac).
            c0 = tt * CB
            nc.tensor.matmul(ps[0:2, 0:2], lhsT=xT0[:, c0:c0 + 2],
                             rhs=xT0[:, c0:c0 + 2], start=True, stop=True)
            nc.tensor.matmul(ps[0:2, 2:4], lhsT=xT1[:, c0:c0 + 2],
                             rhs=xT1[:, c0:c0 + 2], start=True, stop=True)
            nc.tensor.matmul(ps[0:2, 4:6], lhsT=xT2[:, c0:c0 + 2],
                             rhs=xT2[:, c0:c0 + 2], start=True, stop=True)
            if tt == 0:
                nc.tensor.matmul(ps[0:4, 6:10], lhsT=sel_sb, rhs=sel_sb,
                                 start=True, stop=True)
                nc.tensor.matmul(ps[0:2, 10:12], lhsT=selT_sb[:, 0:2],
                                 rhs=selT_sb[:, 0:2], start=True, stop=True)
                nc.tensor.matmul(ps[0:2, 12:14], lhsT=ones1[:, 0:2],
                                 rhs=ones1[:, 0:2], start=True, stop=True)
                nc.tensor.matmul(ps[0:2, 14:16], lhsT=perm_sb[:, 0:2],
                                 rhs=perm_sb[:, 0:2], start=True, stop=True)
            for n in range(CB // 512):
                c512 = slice(tt * CB + n * 512, tt * CB + n * 512 + 512)
                pss = ps[:, n * 512:(n + 1) * 512]
                nc.tensor.matmul(pss, lhsT=w8a, rhs=xT0[:, c512],
                                 start=True, stop=False)
                nc.tensor.matmul(pss, lhsT=w8b, rhs=xT1[:, c512],
                                 start=False, stop=False)
                nc.tensor.matmul(pss, lhsT=w8c, rhs=xT2[:, c512],
                                 start=False, stop=True)
            # evacuate rows 0:36 + bias in one ACT op (psA has one reader);
            # gate rows sit at 0:4 (fwd) and 32:36 (bwd) - engine reads must
            # start at partition 0/32/64/96.
            st8 = stage.tile([36, CB], F16, tag="st8")
            nc.scalar.activation(st8, ps, AF.Identity, bias=b8_sb[:, :],
                                 scale=1.0)
            dst0 = W + toff
            nc.sync.dma_start(out=dxg[b * 8:b * 8 + 4, dst0:dst0 + CB],
                              in_=st8[0:4, :])
            # d=1 rows stored time-REVERSED (col W+r holds t=T-1-r): flip on
            # DVE (fp16 4x) so the DMA writes contiguous runs.
            strev = stage.tile([4, CB], F16, tag="strev")
            nc.vector.tensor_copy(strev, st8[32:36, ::-1])
            lo = PADROW - CB - dst0
            nc.sync.dma_start(out=dxg[b * 8 + 4:b * 8 + 8, lo:lo + CB],
                              in_=strev)
            if toff + CB == T:
                # batch b complete: gather both directions into scan layout.
                # dst rows d*64 + b*16 + k (contiguous 16-block); src runs of
                # S elems, chunk k at col k*L (d0) / (K-1-k)*L (d1, reversed
                # storage makes in-run order forward).
                base = dxg[:, :]
                src_f = bass.AP(
                    tensor=base.tensor, offset=(b * 8) * PADROW,
                    ap=[[L, K], [PADROW, 4], [1, S]])
                nc.sync.dma_start(
                    out=xg_tile[b * 16:(b + 1) * 16, :].rearrange(
                        "p (g s) -> p g s", g=4),
                    in_=src_f)
                # ascending window index khat: row 64+b*16+khat holds scan
                # chunk k = K-1-khat; att pairing fixed later by a PE permute.
                src_b = bass.AP(
                    tensor=base.tensor,
                    offset=(b * 8 + 4) * PADROW,
                    ap=[[L, K], [PADROW, 4], [1, S]])
                nc.sync.dma_start(
                    out=xg_tile[64 + b * 16:64 + (b + 1) * 16, :].rearrange(
                        "p (g s) -> p g s", g=4),
                    in_=src_b)

        p1ctx.close()
        scanctx = ExitStack()
        scanp = scanctx.enter_context(tc.tile_pool(name="scanp", bufs=1))
        psB = scanctx.enter_context(
            tc.tile_pool(name="psB", bufs=1, space="PSUM"))

        # ---- phase 3: fixed-point iterations ----
        h_st = singles.tile([128, S + 1], F16)   # col 0 stays zero
        nc.vector.memset(h_st[:, :], 0.0)
        gbuf = scanp.tile([128, 4 * S], F16, tag="gbuf")
        St = scanp.tile([128, 3 * S], F16, tag="St")
        Gt = scanp.tile([128, S], F16, tag="Gt")
        mt = scanp.tile([128, S], F16, tag="mt")
        ct = scanp.tile([128, S], F16, tag="ct")
        tct = scanp.tile([128, S], F16, tag="tct")
        # gate order in xg_tile cols: 0=i, 1=f, 2=o, 3=g
        for it in range(N_ITER):
            gsrc = xg_tile if it == 0 else gbuf
            if it > 0:
                for g in (0, 3, 1, 2):   # i, g first: unblocks mt earliest
                    nc.vector.scalar_tensor_tensor(
                        out=gbuf[:, g * S:(g + 1) * S],
                        in0=h_st[:, 0:S],
                        scalar=whh_sb[:, g:g + 1],
                        in1=xg_tile[:, g * S:(g + 1) * S],
                        op0=ALU.mult, op1=ALU.add)
            nc.scalar.activation(St[:, 0:S], gsrc[:, 0:S], AF.Sigmoid)
            nc.scalar.activation(Gt, gsrc[:, 3 * S:4 * S], AF.Tanh)
            nc.vector.tensor_mul(mt, St[:, 0:S], Gt)
            nc.scalar.activation(St[:, S:2 * S], gsrc[:, S:2 * S], AF.Sigmoid)
            nc.vector.tensor_tensor_scan(
                out=ct, data0=St[:, S:2 * S], data1=mt, initial=0.0,
                op0=ALU.mult, op1=ALU.add)
            nc.scalar.activation(St[:, 2 * S:3 * S], gsrc[:, 2 * S:3 * S],
                                 AF.Sigmoid)
            nc.scalar.activation(tct, ct, AF.Tanh)
            nc.vector.tensor_mul(h_st[:, 1:S + 1], St[:, 2 * S:3 * S], tct)

        # ---- phase 4: attention ----
        # backward h: inner flip on DVE, then PE permutes rows khat -> K-1-k
        # within each batch block (partition permutation via matmul).
        h_rev = scanp.tile([64, S + 1], F16, tag="hrev")
        nc.vector.tensor_copy(h_rev, h_st[64:128, ::-1])
        hb_perm = psB.tile([64, L], F32, tag="hbp")
        nc.tensor.matmul(hb_perm, lhsT=perm_sb, rhs=h_rev[:, 0:L],
                         start=True, stop=True)
        hsum = scanp.tile([64, L], F16, tag="hsum")
        nc.vector.tensor_add(hsum, h_st[0:64, W + 1:S + 1], hb_perm)
        # logits = 0.5*hsum with hsum in (-2,2): exp(0.5*hsum - 1) in
        # [e^-2, 1], so no max-subtraction needed.
        negone = scanp.tile([64, 1], F32, tag="negone")
        nc.vector.memset(negone[:, :], -1.0)
        exps = scanp.tile([64, L], F32, tag="exps")
        s1 = scanp.tile([64, 1], F32, tag="s1")
        nc.scalar.activation(exps, hsum, AF.Exp, bias=negone[:, :], scale=0.5,
                             accum_out=s1)
        ps_s = psB.tile([4, 1], F32, tag="pss")
        nc.tensor.matmul(ps_s, lhsT=sel_sb, rhs=s1, start=True, stop=True)
        r4 = scanp.tile([4, 1], F32, tag="r4")
        nc.vector.reciprocal(r4, ps_s)
        ps_r = psB.tile([64, 1], F32, tag="psr")
        nc.tensor.matmul(ps_r, lhsT=selT_sb, rhs=r4, start=True, stop=True)
        att_r = scanp.tile([64, L], F16, tag="attr")
        nc.vector.tensor_scalar_mul(att_r, exps, ps_r[:, 0:1])
        # flatten to token order in a single partition: row r=(b*16+k) lands
        # at offset r*L, i.e. datt_row[0, b*4096 + k*256 + s].
        datt_row = singles.tile([1, TOK], F16)
        nc.sync.dma_start(
            out=datt_row[0:1, :].rearrange("p (r s) -> p r s", r=64),
            in_=att_r[:, :])

        scanctx.close()
        p5ctx = ExitStack()
        papool = p5ctx.enter_context(tc.tile_pool(name="papool", bufs=2))
        opool = p5ctx.enter_context(tc.tile_pool(name="opool", bufs=4))
        psP = p5ctx.enter_context(tc.tile_pool(name="psP", bufs=2,
                                               space="PSUM"))

        # ---- phase 5: out_T = xT * att ----
        # Broadcast att across 128 partitions with a K=1 PE outer product
        # (ones ⊗ att_row) into PSUM, evacuate on ACT; DVE multiplies.
        for tt in range(NB):
            cols = slice(tt * CB, (tt + 1) * CB)
            pp = psP.tile([128, CB], F32, tag="pp")
            if tt == 0:
                nc.tensor.matmul(pp[0:2, 0:2], lhsT=datt_row[:, 0:2],
                                 rhs=datt_row[:, 0:2], start=True, stop=True)
            for j in range(CB // 512):
                nc.tensor.matmul(
                    pp[:, j * 512:(j + 1) * 512], lhsT=ones1,
                    rhs=datt_row[:, tt * CB + j * 512:tt * CB + (j + 1) * 512],
                    start=True, stop=True)
            pa = papool.tile([128, CB], F16, tag="pa")
            nc.scalar.activation(pa, pp, AF.Identity)
            ob0 = opool.tile([128, CB], F16, tag="ob")
            nc.vector.tensor_mul(ob0, xT0[:, cols], pa)
            nc.sync.dma_start(out=outT[0:128, cols], in_=ob0)
            ob1 = opool.tile([128, CB], F16, tag="ob")
            nc.vector.tensor_mul(ob1, xT1[:, cols], pa)
            nc.scalar.dma_start(out=outT[128:256, cols], in_=ob1)
            ob2 = opool.tile([44, CB], F16, tag="ob2")
            nc.gpsimd.tensor_mul(ob2, xT2[:, cols], pa[0:44, :])
            nc.sync.dma_start(out=outT[256:300, cols], in_=ob2)
        p5ctx.close()

    return nc


_NC = None


def _get_nc():
    global _NC
    if _NC is None:
        _NC = _build_nc()
        _NC.finalize()
    return _NC


def _prep_core_inputs(x, w_ih_f, w_hh_f, b_ih_f, b_hh_f,
                      w_ih_b, w_hh_b, b_ih_b, b_hh_b):
    """Build the per-core input maps."""
    w8T = np.zeros((E, 36), np.float32)
    b8 = np.zeros((36, 1), np.float32)
    whh = np.zeros((P, 4), np.float32)
    for d, (wi, wh, bi, bh) in enumerate(
            [(w_ih_f, w_hh_f, b_ih_f, b_hh_f),
             (w_ih_b, w_hh_b, b_ih_b, b_hh_b)]):
        for j, gp in enumerate(GATE_PERM):
            w8T[:, d * 32 + j] = wi[gp, :]
            b8[d * 32 + j, 0] = bi[gp] + bh[gp]
            whh[d * 64:(d + 1) * 64, j] = wh[gp, 0]
    sel = np.zeros((64, 4), np.float32)
    for r in range(64):
        sel[r, r // 16] = 1.0
    selT = np.ascontiguousarray(sel.T)
    permM = np.zeros((64, 64), np.float16)
    for bb in range(4):
        for i in range(16):
            permM[bb * 16 + i, bb * 16 + 15 - i] = 1.0
    w8T16 = w8T.astype(np.float16)

    maps = []
    for c in range(NCORES):
        xs = x[c * BL:(c + 1) * BL]                       # [4, T, E]
        xTc = np.ascontiguousarray(
            xs.transpose(2, 0, 1).reshape(E, TOK)).astype(np.float16)
        maps.append({"xT": xTc, "w8T": w8T16, "b8": b8, "whh": whh,
                     "sel": sel, "selT": selT, "permM": permM})
    return maps


def _run(inputs, trace=False, tmpdir=None):
    nc = _get_nc()
    maps = _prep_core_inputs(**inputs)
    res = run_bass_kernel_spmd(nc, maps, list(range(NCORES)), trace=trace,
                               tmpdir=tmpdir)
    outs = []
    for c in range(NCORES):
        oT = res.results[c]["outT"].astype(np.float32)    # [E, TOK]
        outs.append(oT.reshape(E, BL, T).transpose(1, 2, 0))
    return np.concatenate(outs, axis=0), res


def kernel(**inputs):
    out, _ = _run(inputs, trace=False)
    return out


# revision 10
# speedup vs baseline: 1.2893x; 1.2893x over previous
"""Bidirectional H=1 LSTM attention kernel for Trainium2 (8 NeuronCores).

Model: hs = BiLSTM(x) [B,T,2] -> att = softmax(mean(hs,-1), axis=T) -> out = att[:,:,None]*x
Shapes: B=32, T=4096, E=300, H=1.

v2: fp16 datapath (memory-regime problem; halves HBM traffic, 4x PE rate):
  - Host converts x to fp16 e-major [300, 16384] per core; out returned fp16
    and upcast on host.  All on-chip tensors fp16 except PSUM (f32), scan
    internal state (f32) and the softmax sum (f32).
  - Phase 1: stream xT blocks (2048 toks), PE computes xg [8,2048] in PSUM
    (fp16 matmuls, 1 cyc/row), ACT evacuates +bias -> fp16 staging, DVE
    flips the bwd rows (time-reversed storage), DMAs -> dxg DRAM.
  - Phase 2: per-(d,b) merged gather DMAs (16 chunk-rows each) -> scan
    layout xg_tile [128 rows = (d,b,k) b-major, 4 gates x S cols].
  - Phase 3: fixed-point halo scan (L=256, W=32, N_ITER iters), fp16 ops
    (DVE 2x packing), fp32 scan state inside tensor_tensor_scan.
  - Phase 4: softmax over T per batch via PE sel-reduction.
  - Phase 5: att broadcast to 128 partitions via PE outer product (no HBM
    broadcast traffic), DVE/gpsimd multiply, fp16 writes.
"""

import sys

sys.path.insert(0, "/opt/trn_rl_repo")

import numpy as np
from contextlib import ExitStack

import concourse.bass as bass
import concourse.bacc as bacc
import concourse.tile as tile
from concourse import mybir
from concourse.bass_utils import run_bass_kernel_spmd

F32 = mybir.dt.float32
USE_BF16 = True
F16 = mybir.dt.bfloat16 if USE_BF16 else mybir.dt.float16
import ml_dtypes
NP16 = ml_dtypes.bfloat16 if USE_BF16 else np.float16
AF = mybir.ActivationFunctionType
ALU = mybir.AluOpType

NCORES = 8
B, T, E = 32, 4096, 300
BL = B // NCORES          # batches per core
TOK = BL * T              # tokens per core (b-major)
L, W = 256, 32            # chunk len, halo warmup
S = L + W                 # scan steps per chunk
K = T // L                # chunks per (dir, batch)
P = 128                   # partitions = d*64 + b*16 + k (b-major)
N_ITER = 4                # fixed-point iterations (validated offline:
                          # N=4 hits the fp16 error floor 2.3e-3; N=3 7e-3)
PADROW = W + T + W        # padded xg row: [0..W) zeros, [W..W+T) data, tail zeros
CB = 2048                 # phase-1/5 token block
NB = TOK // CB
# gate order inside a block row: (i, f, o, g) ; pytorch order is (i, f, g, o)
GATE_PERM = [0, 1, 3, 2]


def _build_nc():
    nc = bacc.Bacc(None, target_bir_lowering=False, debug=False)
    xT = nc.declare_dram_parameter("xT", [E, TOK], F16, isOutput=False)
    w8T = nc.declare_dram_parameter("w8T", [E, 36], F16, isOutput=False)
    b8 = nc.declare_dram_parameter("b8", [36, 1], F32, isOutput=False)
    whh = nc.declare_dram_parameter("whh", [P, 4], F32, isOutput=False)
    sel = nc.declare_dram_parameter("sel", [64, 4], F32, isOutput=False)
    selT = nc.declare_dram_parameter("selT", [4, 64], F32, isOutput=False)
    permM = nc.declare_dram_parameter("permM", [64, 64], F16, isOutput=False)
    outT = nc.declare_dram_parameter("outT", [E, TOK], F16, isOutput=True)

    # internal DRAM scratch: rows b*8 + d*4 + g
    dxg = nc.dram_tensor("dxg", [32, PADROW], F16)

    with tile.TileContext(nc) as tc, ExitStack() as ctx:
        singles = ctx.enter_context(tc.tile_pool(name="singles", bufs=1))
        p1ctx = ExitStack()
        stage = p1ctx.enter_context(tc.tile_pool(name="stage", bufs=2))
        psA = p1ctx.enter_context(tc.tile_pool(name="psA", bufs=2, space="PSUM"))

        # ---- constants / resident tiles ----
        w8a = singles.tile([128, 36], F16)
        w8b = singles.tile([128, 36], F16)
        w8c = singles.tile([44, 36], F16)
        nc.gpsimd.dma_start(out=w8a, in_=w8T[0:128, :])
        nc.gpsimd.dma_start(out=w8b, in_=w8T[128:256, :])
        nc.gpsimd.dma_start(out=w8c, in_=w8T[256:300, :])
        b8_sb = singles.tile([36, 1], F32)
        nc.sync.dma_start(out=b8_sb, in_=b8[:, :])
        whh_sb = singles.tile([P, 4], F32)
        nc.sync.dma_start(out=whh_sb, in_=whh[:, :])
        sel_sb = singles.tile([64, 4], F32)
        nc.sync.dma_start(out=sel_sb, in_=sel[:, :])
        selT_sb = singles.tile([4, 64], F32)
        nc.sync.dma_start(out=selT_sb, in_=selT[:, :])
        perm_sb = singles.tile([64, 64], F16)
        nc.sync.dma_start(out=perm_sb, in_=permM[:, :])
        ones1 = singles.tile([1, 128], F16)
        nc.vector.memset(ones1[:, :], 1.0)

        xT0 = singles.tile([128, TOK], F16)   # e 0..127 resident
        xT1 = singles.tile([128, TOK], F16)   # e 128..255 resident
        xT2 = singles.tile([44, TOK], F16)    # e 256..299 resident

        # zero-pad regions of dxg (halo reads beyond sequence ends)
        zpad = singles.tile([32, W], F16)
        nc.vector.memset(zpad[:, :], 0.0)
        nc.sync.dma_start(out=dxg[:, 0:W], in_=zpad[:, :])
        nc.sync.dma_start(out=dxg[:, W + T:PADROW], in_=zpad[:, :])

        # scan-layout xg: rows p = d*64 + b*16 + k, cols g*S + s
        xg_tile = singles.tile([128, 4 * S], F16)

        # ---- phase 1: stream xT, compute xg -> dxg; gather per (d,b) ----
        for tt in range(NB):
            b = (tt * CB) // T
            toff = (tt * CB) % T
            cols = slice(tt * CB, (tt + 1) * CB)
            d0 = nc.gpsimd.dma_start(out=xT0[:, cols], in_=xT[0:128, cols])
            d1 = nc.sync.dma_start(out=xT1[:, cols], in_=xT[128:256, cols])
            d2 = nc.scalar.dma_start(out=xT2[:, cols], in_=xT[256:300, cols])
            ps = psA.tile([36, CB], F32, tag="ps")
            # Matmult codegen has a single sync-wait slot; absorb each fresh
            # DMA's semaphore with a tiny touch matmul reading only that
            # tensor, so real matmuls only wait on the PSUM WAR (ACT evac).
            c0 = tt * CB
            nc.tensor.matmul(ps[0:2, 0:2], lhsT=xT0[:, c0:c0 + 2],
                             rhs=xT0[:, c0:c0 + 2], start=True, stop=True)
            nc.tensor.matmul(ps[0:2, 2:4], lhsT=xT1[:, c0:c0 + 2],
                             rhs=xT1[:, c0:c0 + 2], start=True, stop=True)
            nc.tensor.matmul(ps[0:2, 4:6], lhsT=xT2[:, c0:c0 + 2],
                             rhs=xT2[:, c0:c0 + 2], start=True, stop=True)
            if tt == 0:
                nc.tensor.matmul(ps[0:4, 6:10], lhsT=sel_sb, rhs=sel_sb,
                                 start=True, stop=True)
                nc.tensor.matmul(ps[0:2, 10:12], lhsT=selT_sb[:, 0:2],
                                 rhs=selT_sb[:, 0:2], start=True, stop=True)
                nc.tensor.matmul(ps[0:2, 12:14], lhsT=ones1[:, 0:2],
                                 rhs=ones1[:, 0:2], start=True, stop=True)
                nc.tensor.matmul(ps[0:2, 14:16], lhsT=perm_sb[:, 0:2],
                                 rhs=perm_sb[:, 0:2], start=True, stop=True)
            # weight-stationary order: all 4 sub-blocks per lhsT chunk, so
            # codegen can elide repeated LDWEIGHTS.
            for ci, (wch, xch) in enumerate(
                    [(w8a, xT0), (w8b, xT1), (w8c, xT2)]):
                for n in range(CB // 512):
                    c512 = slice(tt * CB + n * 512, tt * CB + n * 512 + 512)
                    pss = ps[:, n * 512:(n + 1) * 512]
                    nc.tensor.matmul(pss, lhsT=wch, rhs=xch[:, c512],
                                     start=(ci == 0), stop=(ci == 2),
                                     skip_group_check=True)
            # evacuate rows 0:36 + bias in one ACT op (psA has one reader);
            # gate rows sit at 0:4 (fwd) and 32:36 (bwd) - engine reads must
            # start at partition 0/32/64/96.
            st8 = stage.tile([36, CB], F16, tag="st8")
            nc.scalar.activation(st8, ps, AF.Identity, bias=b8_sb[:, :],
                                 scale=1.0)
            dst0 = W + toff
            nc.sync.dma_start(out=dxg[b * 8:b * 8 + 4, dst0:dst0 + CB],
                              in_=st8[0:4, :])
            # d=1 rows stored time-REVERSED (col W+r holds t=T-1-r): flip on
            # DVE (fp16 4x) so the DMA writes contiguous runs.
            strev = stage.tile([4, CB], F16, tag="strev")
            nc.vector.tensor_copy(strev, st8[32:36, ::-1])
            lo = PADROW - CB - dst0
            nc.sync.dma_start(out=dxg[b * 8 + 4:b * 8 + 8, lo:lo + CB],
                              in_=strev)
            if toff + CB == T:
                # batch b complete: gather both directions into scan layout.
                # dst rows d*64 + b*16 + k (contiguous 16-block); src runs of
                # S elems, chunk k at col k*L (d0) / (K-1-k)*L (d1, reversed
                # storage makes in-run order forward).
                base = dxg[:, :]
                src_f = bass.AP(
                    tensor=base.tensor, offset=(b * 8) * PADROW,
                    ap=[[L, K], [PADROW, 4], [1, S]])
                nc.sync.dma_start(
                    out=xg_tile[b * 16:(b + 1) * 16, :].rearrange(
                        "p (g s) -> p g s", g=4),
                    in_=src_f)
                # ascending window index khat: row 64+b*16+khat holds scan
                # chunk k = K-1-khat; att pairing fixed later by a PE permute.
                src_b = bass.AP(
                    tensor=base.tensor,
                    offset=(b * 8 + 4) * PADROW,
                    ap=[[L, K], [PADROW, 4], [1, S]])
                nc.sync.dma_start(
                    out=xg_tile[64 + b * 16:64 + (b + 1) * 16, :].rearrange(
                        "p (g s) -> p g s", g=4),
                    in_=src_b)

        p1ctx.close()
        scanctx = ExitStack()
        scanp = scanctx.enter_context(tc.tile_pool(name="scanp", bufs=1))
        psB = scanctx.enter_context(
            tc.tile_pool(name="psB", bufs=1, space="PSUM"))

        # ---- phase 3: fixed-point iterations ----
        h_st = singles.tile([128, S + 1], F16)   # col 0 stays zero
        nc.vector.memset(h_st[:, :], 0.0)
        gbuf = scanp.tile([128, 4 * S], F16, tag="gbuf")
        St = scanp.tile([128, 3 * S], F16, tag="St")
        Gt = scanp.tile([128, S], F16, tag="Gt")
        mt = scanp.tile([128, S], F16, tag="mt")
        ct = scanp.tile([128, S], F16, tag="ct")
        tct = scanp.tile([128, S], F16, tag="tct")
        # gate order in xg_tile cols: 0=i, 1=f, 2=o, 3=g
        for it in range(N_ITER):
            gsrc = xg_tile if it == 0 else gbuf
            if it > 0:
                for g in (0, 3, 1, 2):   # i, g first: unblocks mt earliest
                    nc.vector.scalar_tensor_tensor(
                        out=gbuf[:, g * S:(g + 1) * S],
                        in0=h_st[:, 0:S],
                        scalar=whh_sb[:, g:g + 1],
                        in1=xg_tile[:, g * S:(g + 1) * S],
                        op0=ALU.mult, op1=ALU.add)
            nc.scalar.activation(St[:, 0:S], gsrc[:, 0:S], AF.Sigmoid)
            nc.scalar.activation(Gt, gsrc[:, 3 * S:4 * S], AF.Tanh)
            nc.vector.tensor_mul(mt, St[:, 0:S], Gt)
            nc.scalar.activation(St[:, S:2 * S], gsrc[:, S:2 * S], AF.Sigmoid)
            nc.vector.tensor_tensor_scan(
                out=ct, data0=St[:, S:2 * S], data1=mt, initial=0.0,
                op0=ALU.mult, op1=ALU.add)
            nc.scalar.activation(St[:, 2 * S:3 * S], gsrc[:, 2 * S:3 * S],
                                 AF.Sigmoid)
            nc.scalar.activation(tct, ct, AF.Tanh)
            nc.vector.tensor_mul(h_st[:, 1:S + 1], St[:, 2 * S:3 * S], tct)

        # ---- phase 4: attention ----
        # backward h: inner flip on DVE, then PE permutes rows khat -> K-1-k
        # within each batch block (partition permutation via matmul).
        h_rev = scanp.tile([64, S + 1], F16, tag="hrev")
        nc.vector.tensor_copy(h_rev, h_st[64:128, ::-1])
        hb_perm = psB.tile([64, L], F32, tag="hbp")
        nc.tensor.matmul(hb_perm, lhsT=perm_sb, rhs=h_rev[:, 0:L],
                         start=True, stop=True)
        hsum = scanp.tile([64, L], F16, tag="hsum")
        nc.vector.tensor_add(hsum, h_st[0:64, W + 1:S + 1], hb_perm)
        # logits = 0.5*hsum with hsum in (-2,2): exp(0.5*hsum - 1) in
        # [e^-2, 1], so no max-subtraction needed.
        negone = scanp.tile([64, 1], F32, tag="negone")
        nc.vector.memset(negone[:, :], -1.0)
        exps = scanp.tile([64, L], F32, tag="exps")
        s1 = scanp.tile([64, 1], F32, tag="s1")
        nc.scalar.activation(exps, hsum, AF.Exp, bias=negone[:, :], scale=0.5,
                             accum_out=s1)
        ps_s = psB.tile([4, 1], F32, tag="pss")
        nc.tensor.matmul(ps_s, lhsT=sel_sb, rhs=s1, start=True, stop=True)
        r4 = scanp.tile([4, 1], F32, tag="r4")
        nc.vector.reciprocal(r4, ps_s)
        ps_r = psB.tile([64, 1], F32, tag="psr")
        nc.tensor.matmul(ps_r, lhsT=selT_sb, rhs=r4, start=True, stop=True)
        att_r = scanp.tile([64, L], F16, tag="attr")
        nc.vector.tensor_scalar_mul(att_r, exps, ps_r[:, 0:1])
        # flatten to token order in a single partition: row r=(b*16+k) lands
        # at offset r*L, i.e. datt_row[0, b*4096 + k*256 + s].
        datt_row = singles.tile([1, TOK], F16)
        nc.sync.dma_start(
            out=datt_row[0:1, :].rearrange("p (r s) -> p r s", r=64),
            in_=att_r[:, :])

        scanctx.close()
        p5ctx = ExitStack()
        papool = p5ctx.enter_context(tc.tile_pool(name="papool", bufs=2))
        opool = p5ctx.enter_context(tc.tile_pool(name="opool", bufs=4))
        psP = p5ctx.enter_context(tc.tile_pool(name="psP", bufs=2,
                                               space="PSUM"))

        # ---- phase 5: out_T = xT * att ----
        # Broadcast att across 128 partitions with a K=1 PE outer product
        # (ones ⊗ att_row) into PSUM, evacuate on ACT; DVE multiplies.
        for tt in range(NB):
            cols = slice(tt * CB, (tt + 1) * CB)
            pp = psP.tile([128, CB], F32, tag="pp")
            if tt == 0:
                nc.tensor.matmul(pp[0:2, 0:2], lhsT=datt_row[:, 0:2],
                                 rhs=datt_row[:, 0:2], start=True, stop=True)
            for j in range(CB // 512):
                nc.tensor.matmul(
                    pp[:, j * 512:(j + 1) * 512], lhsT=ones1,
                    rhs=datt_row[:, tt * CB + j * 512:tt * CB + (j + 1) * 512],
                    start=True, stop=True)
            pa = papool.tile([128, CB], F16, tag="pa")
            nc.scalar.activation(pa, pp, AF.Identity)
            ob0 = opool.tile([128, CB], F16, tag="ob")
            nc.vector.tensor_mul(ob0, xT0[:, cols], pa)
            nc.sync.dma_start(out=outT[0:128, cols], in_=ob0)
            ob1 = opool.tile([128, CB], F16, tag="ob")
            nc.vector.tensor_mul(ob1, xT1[:, cols], pa)
            nc.scalar.dma_start(out=outT[128:256, cols], in_=ob1)
            ob2 = opool.tile([44, CB], F16, tag="ob2")
            nc.vector.tensor_mul(ob2, xT2[:, cols], pa[0:44, :])
            nc.scalar.dma_start(out=outT[256:300, cols], in_=ob2)
        p5ctx.close()

    return nc


_NC = None


def _get_nc():
    global _NC
    if _NC is None:
        _NC = _build_nc()
        _NC.finalize()
    return _NC


def _prep_core_inputs(x, w_ih_f, w_hh_f, b_ih_f, b_hh_f,
                      w_ih_b, w_hh_b, b_ih_b, b_hh_b):
    """Build the per-core input maps."""
    w8T = np.zeros((E, 36), np.float32)
    b8 = np.zeros((36, 1), np.float32)
    whh = np.zeros((P, 4), np.float32)
    for d, (wi, wh, bi, bh) in enumerate(
            [(w_ih_f, w_hh_f, b_ih_f, b_hh_f),
             (w_ih_b, w_hh_b, b_ih_b, b_hh_b)]):
        for j, gp in enumerate(GATE_PERM):
            w8T[:, d * 32 + j] = wi[gp, :]
            b8[d * 32 + j, 0] = bi[gp] + bh[gp]
            whh[d * 64:(d + 1) * 64, j] = wh[gp, 0]
    sel = np.zeros((64, 4), np.float32)
    for r in range(64):
        sel[r, r // 16] = 1.0
    selT = np.ascontiguousarray(sel.T)
    permM = np.zeros((64, 64), NP16)
    for bb in range(4):
        for i in range(16):
            permM[bb * 16 + i, bb * 16 + 15 - i] = 1.0
    w8T16 = w8T.astype(NP16)

    maps = []
    for c in range(NCORES):
        xs = x[c * BL:(c + 1) * BL]                       # [4, T, E]
        xTc = np.ascontiguousarray(
            xs.transpose(2, 0, 1).reshape(E, TOK)).astype(NP16)
        maps.append({"xT": xTc, "w8T": w8T16, "b8": b8, "whh": whh,
                     "sel": sel, "selT": selT, "permM": permM})
    return maps


def _run(inputs, trace=False, tmpdir=None):
    nc = _get_nc()
    maps = _prep_core_inputs(**inputs)
    res = run_bass_kernel_spmd(nc, maps, list(range(NCORES)), trace=trace,
                               tmpdir=tmpdir)
    outs = []
    for c in range(NCORES):
        oT = res.results[c]["outT"].astype(np.float32)    # [E, TOK]
        outs.append(oT.reshape(E, BL, T).transpose(1, 2, 0))
    return np.concatenate(outs, axis=0), res


def kernel(**inputs):
    out, _ = _run(inputs, trace=False)
    return out
